# revision 52
# baseline (speedup 1.0000x reference)
"""Trainium2 Bass kernel for nn_DepthWiseConv_AConnect (depthwise 3x3 conv with
per-pool multiplicative weight/bias noise, followed by 8-bit LQuant).

Strategy (8 NeuronCores, data-parallel over the pool axis):
  - Core p handles pool group p: 8 images, Werr[p], Berr[p].
  - The conv is split across FOUR engines by output-row bands so no single
    engine is the bottleneck:
      * PE rows: 9 accumulating diagonal matmuls per PSUM tile
        (psum[c, pix] += diag(127*w_t)[c] * X^T[c, pix + shift_t]);
        ScalarE evacuates PSUM + bias directly to int8 (HW cast is
        round-to-nearest-even with saturation == the lquant grid).
      * DVE rows: 2 taps as fused scalar_tensor_tensor (bias folded into
        the first), 7 taps as product passes (4 on ScalarE, 3 on GpSimd
        tensor_scalar) + DVE tensor_tensor adds at 2x; final per-image
        tensor_scalar pass (*127 -> int8). All vector-engine ops use <=3D
        access patterns (the HW verifier rejects 4D on TensorScalar ops).
  - PE warm-up: dummy matmuls during the initial DMA keep the first real
    matmuls at 2.4 GHz (HAM activity window).
  - Inputs ship fp16 (exact enough in fp32/fp16 accumulation), outputs int8.
  - Host clamps int8 -128 -> -127 (lquant clips at -127) and divides by 127.
"""
import sys

import numpy as np

try:
    import concourse.bacc as bacc_mod
except ImportError:
    sys.path.insert(0, "/opt/trn_rl_repo")
    import concourse.bacc as bacc_mod

import concourse.mybir as mybir
from concourse.tile import TileContext
from concourse.bass_utils import run_bass_kernel_spmd
from contextlib import ExitStack

POOL = 8
NB = 8            # images per pool group (64 / 8)
H = W = 56
HO = WO = 54
C = 256
NCH = 2           # channel chunks of 128
NPIX = H * W      # 3136
NOUT = HO * WO    # 2916
S = 127.0
GRP = 4           # images batched per vector-engine op
NGRP = NB // GRP

# Output-row split (rows_pe, rows_dve, rows_pool) per macro-chunk. The last
# macro is PE-heavy so the slower vector engines are not the tail.
SPLITS = [(32, 22, 0), (33, 21, 0), (33, 21, 0), (42, 12, 0)]
STT_TAPS = 2      # DVE fused taps (incl bias init)
ACT_PRODS = 4     # product taps on ScalarE (4-image batched)
POOL_PRODS = 3    # product taps on GpSimd (per-image, tensor_scalar)
WARMUP_MM = 30

f32 = mybir.dt.float32
f16 = mybir.dt.float16
i8 = mybir.dt.int8
Alu = mybir.AluOpType
Act = mybir.ActivationFunctionType

_cached = {}


def _pe_tiles(rows):
    tiles, r = [], 0
    while r < rows:
        n = min(9, rows - r)
        tiles.append((r, n))
        r += n
    return tiles


def _build():
    nc = bacc_mod.Bacc()
    xt = nc.dram_tensor("xt", [NGRP, NCH, 128, GRP, NPIX], f16, kind="ExternalInput")
    wdg = nc.dram_tensor("wdg", [128, NCH, 9, 128], f16, kind="ExternalInput")
    wv = nc.dram_tensor("wv", [128, NCH, 9], f32, kind="ExternalInput")
    bvs = nc.dram_tensor("bvs", [128, NCH], f32, kind="ExternalInput")  # *127
    bv = nc.dram_tensor("bv", [128, NCH], f32, kind="ExternalInput")    # unscaled
    out = nc.dram_tensor("out", [NGRP, NCH, 128, GRP, NOUT], i8,
                         kind="ExternalOutput")

    with TileContext(nc) as tc, ExitStack() as ctx:
        consts = ctx.enter_context(tc.tile_pool(name="consts", bufs=1))
        xpool = ctx.enter_context(tc.tile_pool(name="xpool", bufs=3))
        vpool = ctx.enter_context(tc.tile_pool(name="vpool", bufs=2))
        prpool = ctx.enter_context(tc.tile_pool(name="prpool", bufs=6))
        opool = ctx.enter_context(tc.tile_pool(name="opool", bufs=2))
        pspool = ctx.enter_context(tc.tile_pool(name="pspool", bufs=7, space="PSUM"))
        wupool = ctx.enter_context(tc.tile_pool(name="wupool", bufs=1, space="PSUM"))

        # PE warm-up: dummy matmuls on a const tile while the first DMAs land,
        # so the HAM activity window has the PE at 2.4 GHz when real matmuls
        # start (a gap resets the ramp, so these must bridge the DMA wait).
        wu = consts.tile([128, 128], f16)
        nc.vector.memset(wu, 0.0)
        wups = wupool.tile([128, 128], f32, tag="wups")
        for _ in range(WARMUP_MM):
            nc.tensor.matmul(wups, lhsT=wu, rhs=wu, start=True,
                             stop=True, skip_group_check=True)

        macros = [(g, q) for g in range(NGRP) for q in range(NCH)]
        # Issue every x-load up front: the SP sequencer blocks on each DMA's
        # wait-semaphore before dispatching the next, so interleaving loads
        # with (late-completing) output stores would stall later loads. The
        # first image's load is also emitted before the weight DMAs so it
        # wins the DMA engines first.
        xs_tiles = []
        ws = consts.tile([128, NCH, 9, 128], f16)
        wvt = consts.tile([128, NCH, 9], f32)
        bst = consts.tile([128, NCH], f32)
        bt = consts.tile([128, NCH], f32)
        for mi, (g, q) in enumerate(macros):
            xs = xpool.tile([128, GRP, NPIX], f16, tag="xs")
            for n in range(GRP):
                if mi == 0 and n == 0:
                    # The very first image gates the first matmuls, which
                    # only need the top rows: land it in three row-chunks
                    # so PE starts ~3 us earlier.
                    for lo, hi in ((0, 14 * W), (14 * W, 28 * W),
                                   (28 * W, NPIX)):
                        nc.sync.dma_start(out=xs[:, n, lo:hi],
                                          in_=xt[g, q, :, n, lo:hi])
                else:
                    nc.sync.dma_start(out=xs[:, n], in_=xt[g, q, :, n])
            xs_tiles.append(xs)
            if mi == 0:
                # Weights/bias ride the Activation HWDGE queue; q=0 weights
                # first (they alone gate the first matmuls).
                nc.scalar.dma_start(out=ws[:, 0], in_=wdg[:, 0])
                nc.scalar.dma_start(out=wvt, in_=wv[:, :, :])
                nc.scalar.dma_start(out=bst, in_=bvs[:, :])
                nc.scalar.dma_start(out=bt, in_=bv[:, :])
                nc.scalar.dma_start(out=ws[:, 1], in_=wdg[:, 1])

        for mi, (g, q) in enumerate(macros):
            rows_pe, rows_dve, rows_pool = SPLITS[mi]
            xs = xs_tiles[mi]
            xr = xs.rearrange("p g (h w) -> p g h w", w=W)
            ot = opool.tile([128, GRP, NOUT], i8, tag="ot")
            orr = ot.rearrange("p g (h w) -> p g h w", w=WO)

            # ---- PE band (diag matmuls + prioritized Act evac) interleaved
            # with the DVE band's Act product passes, so evacs never queue
            # behind a burst of long multiply passes on the Act engine.
            rv0 = rows_pe
            if rows_dve:
                acc = vpool.tile([128, GRP, rows_dve, WO], f16, tag="acc")
                accf = acc.rearrange("p g r w -> p (g r w)")
            else:
                acc = accf = None

            prods = []

            def av_prod(t):
                i, j = divmod(t, 3)
                xv = xr[:, :, rv0 + i: rv0 + i + rows_dve, j: j + WO]
                pr = prpool.tile([128, GRP, rows_dve, WO], f16, tag="pr")
                nc.scalar.activation(out=pr, in_=xv, func=Act.Identity,
                                     bias=0.0, scale=wvt[:, q, t: t + 1])
                prods.append(pr)

            def pool_prod(t):
                i, j = divmod(t, 3)
                pr = prpool.tile([128, GRP, rows_dve, WO], f16, tag="pr")
                for n in range(GRP):
                    xv = xr[:, n, rv0 + i: rv0 + i + rows_dve, j: j + WO]
                    nc.gpsimd.tensor_scalar(
                        out=pr[:, n], in0=xv, scalar1=wvt[:, q, t: t + 1],
                        scalar2=None, op0=Alu.mult)
                prods.append(pr)

            # The self-contained STT taps run first (with the bias folded
            # into the initial tap), so DVE starts as soon as x lands rather
            # than waiting for products.
            if rows_dve:
                for t in range(STT_TAPS):
                    i, j = divmod(t, 3)
                    for n in range(GRP):
                        xv = xr[:, n, rv0 + i: rv0 + i + rows_dve, j: j + WO]
                        in1 = (bt[:, q: q + 1]
                               .broadcast_to([128, rows_dve, WO])
                               if t == 0 else acc[:, n])
                        nc.vector.scalar_tensor_tensor(
                            out=acc[:, n], in0=xv,
                            scalar=wvt[:, q, t: t + 1], in1=in1,
                            op0=Alu.mult, op1=Alu.add)
                # GpSimd products: ready as soon as each image lands.
                for t in range(STT_TAPS, STT_TAPS + POOL_PRODS):
                    pool_prod(t)

            av_lo = STT_TAPS + POOL_PRODS
            av_hi = av_lo + (ACT_PRODS if rows_dve else 0)
            av_next = av_lo
            if rows_dve and av_next < av_hi:
                av_prod(av_next)
                av_next += 1
            for n in range(GRP):
                for (r0, nr) in _pe_tiles(rows_pe):
                    ps = pspool.tile([128, nr * WO], f32, tag="ps")
                    for t in range(9):
                        i, j = divmod(t, 3)
                        rhs = xr[:, n, r0 + i: r0 + i + nr, j: j + WO]
                        nc.tensor.matmul(ps, lhsT=ws[:, q, t, :], rhs=rhs,
                                         start=(t == 0), stop=(t == 8),
                                         skip_group_check=True)
                    with tc.high_priority(offset=40):
                        nc.scalar.activation(
                            out=ot[:, n, r0 * WO: (r0 + nr) * WO], in_=ps,
                            func=Act.Identity, bias=bst[:, q: q + 1], scale=1.0)
                if av_next < av_hi:
                    av_prod(av_next)
                    av_next += 1
            while av_next < av_hi:
                av_prod(av_next)
                av_next += 1

            if rows_dve:
                for pr in prods:
                    nc.vector.tensor_tensor(
                        out=accf, in0=accf,
                        in1=pr.rearrange("p g r w -> p (g r w)"), op=Alu.add)
                for n in range(GRP):
                    nc.gpsimd.tensor_scalar(
                        out=ot[:, n, rv0 * WO: (rv0 + rows_dve) * WO],
                        in0=acc[:, n].rearrange("p r w -> p (r w)"),
                        scalar1=S, scalar2=None, op0=Alu.mult)

            # ---- Pool band: 9 STT taps + cast pass on GpSimd ----
            r0 = rows_pe + rows_dve
            if rows_pool:
                pacc = ppool.tile([128, GRP, rows_pool, WO], f32, tag="pacc")
                for t in range(9):
                    i, j = divmod(t, 3)
                    for n in range(GRP):
                        xv = xr[:, n, r0 + i: r0 + i + rows_pool, j: j + WO]
                        in1 = (bt[:, q: q + 1]
                               .broadcast_to([128, rows_pool, WO])
                               if t == 0 else pacc[:, n])
                        nc.gpsimd.scalar_tensor_tensor(
                            out=pacc[:, n], in0=xv,
                            scalar=wvt[:, q, t: t + 1], in1=in1,
                            op0=Alu.mult, op1=Alu.add)
                for n in range(GRP):
                    nc.gpsimd.tensor_scalar(
                        out=ot[:, n, r0 * WO: (r0 + rows_pool) * WO],
                        in0=pacc[:, n].rearrange("p r w -> p (r w)"),
                        scalar1=S, scalar2=None, op0=Alu.mult)

            for n in range(GRP):
                if mi == len(macros) - 1 and n >= GRP - 2:
                    # Last image of the run: stream the store out in pieces
                    # as each PSUM evacuation lands instead of waiting for
                    # the whole row.
                    for lo, hi in ((0, 18 * WO), (18 * WO, 36 * WO),
                                   (36 * WO, NOUT)):
                        nc.sync.dma_start(out=out[g, q, :, n, lo:hi],
                                          in_=ot[:, n, lo:hi])
                else:
                    nc.sync.dma_start(out=out[g, q, :, n], in_=ot[:, n])

    nc.finalize()
    return nc


def kernel(X, W, bias, Werr, Berr, _trace=False):
    X = np.asarray(X, np.float32)
    W = np.asarray(W, np.float32)
    bias = np.asarray(bias, np.float32)
    Werr = np.asarray(Werr, np.float32)
    Berr = np.asarray(Berr, np.float32)

    if "nc" not in _cached:
        _cached["nc"] = _build()
    nc = _cached["nc"]

    Xh = X.astype(np.float16)  # [64, 56, 56, 256]
    w3 = W[..., 0]             # [3, 3, 256]
    we3 = Werr[..., 0]         # [8, 3, 3, 256]

    in_maps = []
    for p in range(POOL):
        xp = Xh[p * NB:(p + 1) * NB].reshape(NGRP, GRP, NPIX, C)
        xp = np.ascontiguousarray(xp.transpose(0, 3, 1, 2))  # [NGRP, C, GRP, NPIX]
        xp = xp.reshape(NGRP, NCH, 128, GRP, NPIX)

        w_eff = (w3 * we3[p]).astype(np.float32)              # [3, 3, 256]
        w_s = (np.float32(S) * w_eff).astype(np.float16)
        wdg = np.zeros((NCH, 9, 128, 128), np.float16)
        for q in range(NCH):
            for t in range(9):
                i, j = divmod(t, 3)
                np.fill_diagonal(wdg[q, t], w_s[i, j, 128 * q:128 * (q + 1)])
        wdg = np.ascontiguousarray(wdg.transpose(2, 0, 1, 3))  # [128,NCH,9,128]
        wv = np.ascontiguousarray(
            w_eff.reshape(9, NCH, 128).transpose(2, 1, 0))    # [128, NCH, 9]

        b_eff = (bias * Berr[p]).astype(np.float32)
        bvs = np.ascontiguousarray((np.float32(S) * b_eff).reshape(NCH, 128).T)
        bv = np.ascontiguousarray(b_eff.reshape(NCH, 128).T)
        in_maps.append({"xt": xp, "wdg": wdg, "wv": wv, "bvs": bvs, "bv": bv})

    res = run_bass_kernel_spmd(nc, in_maps, core_ids=list(range(POOL)),
                               trace=_trace)
    if _trace:
        _cached["last_result"] = res

    outs = []
    for p in range(POOL):
        o = res.results[p]["out"]  # [NGRP, NCH, 128, GRP, NOUT] int8
        o = np.maximum(o.astype(np.float32), np.float32(-S)) / np.float32(S)
        o = o.reshape(NGRP, C, GRP, NOUT).transpose(0, 2, 3, 1)  # [NGRP,GRP,NOUT,C]
        outs.append(o.reshape(NB, HO, WO, C))
    return np.ascontiguousarray(np.concatenate(outs, axis=0).astype(np.float32))
